# revision 1
# baseline (speedup 1.0000x reference)
"""Trainium2 Bass kernel for nn_DDCModel (DDC trajectory filter).

Math (per trajectory b, L sequential steps):
    X_0 = one_hot(init_states[b])                      # [S] distribution
    r_t = X_t . R[a_{b,t}]                             # reward (output)
    X_{t+1} = X_t @ T[a_{b,t}]                         # [S] x [S,S] matvec

Strategy (8 NeuronCores):
  - T is sharded over the output-state axis: core r owns T[:, :, r*512:(r+1)*512]
    cast to fp16 and kept SBUF-resident ([128, A*KT*512] tile layout).
  - Each step every core computes its 512-wide slice of the next interface for
    all 8 trajectories and all 4 actions in one PE pass: the per-action
    stationaries are action-masked copies of X^T ("xhat"), so the PSUM
    accumulation performs the action selection for free. The 4 actions run
    concurrently on disjoint PE column groups (tile_position), quadrupling the
    effective moving-operand bandwidth.
  - The 512-slices are exchanged with a per-step AllGather (fp16, 8KB/rank),
    and one 3D xbar DMA-transpose rebuilds X^T [128, (nt, r*8+b)] in SBUF.
  - Rewards are a second tiny PE pass (moving = R columns, N=1) that
    accumulates into a dedicated PSUM bank (one column per step, evacuated
    once at the end); it also keeps the PE busy during the collective so the
    HAM clock gate stays open. Every core computes the same full rewards,
    so no final gather is needed.

Host-side: actions/init_states are compile-time data - they become the
one-hot mask stream and the initial X^T tile; no dynamic control flow on
device.
"""
import sys

sys.path.insert(0, "/opt/trn_rl_repo")

import numpy as np

N_CORES = 8
B = 8          # trajectories
A = 4          # actions
S = 4096       # state-space size
L = 128        # trajectory length
NS = S // N_CORES       # 512: per-core output-state slice
KT = S // 128           # 32: contraction k-tiles
NT = NS // 128          # 4: per-core n-tiles

_CACHE = {}


def _build(l_steps: int, variant: str = "full", n_repeat: int = 1, n_junk: int = 0):
    from concourse import bass, tile
    from concourse.bass import mybir

    F32 = mybir.dt.float32
    F16 = mybir.dt.float16

    nc = bass.Bass(num_devices=N_CORES)

    t_tiles = nc.declare_dram_parameter("t_tiles", [128, A * KT * NS], F16, isOutput=False)
    r_tiles = nc.declare_dram_parameter("r_tiles", [128, A * KT], F16, isOutput=False)
    x0t = nc.declare_dram_parameter("x0t", [128, NT * 64], F16, isOutput=False)
    masks = nc.declare_dram_parameter("masks", [l_steps, 128, A * NT * 64], F16, isOutput=False)
    out = nc.declare_dram_parameter("out", [B, l_steps], F32, isOutput=True)

    cc_in = [nc.dram_tensor(f"cc_in{i}", [B, NS], F16) for i in range(2)]
    cc_out = [
        nc.dram_tensor(f"cc_out{i}", [N_CORES * B, NS], F16, addr_space="Shared")
        for i in range(2)
    ]
    xabs = nc.dram_tensor("xabs", [1, 8], F16)

    with tile.TileContext(nc) as tc:
        with tc.tile_pool(name="const", bufs=1) as cp, \
             tc.tile_pool(name="loop", bufs=3) as lp, \
             tc.tile_pool(name="ps", bufs=2, space="PSUM") as pmp, \
             tc.tile_pool(name="psj", bufs=2, space="PSUM") as pjp, \
             tc.tile_pool(name="psr", bufs=1, space="PSUM") as prp:

            # ---- resident tensors ----
            t_sb = cp.tile([128, A * KT * NS], F16, tag="t_sb")
            nc.sync.dma_start(out=t_sb[:], in_=t_tiles[:])
            r_sb = cp.tile([128, A * KT], F16, tag="r_sb")
            nc.sync.dma_start(out=r_sb[:], in_=r_tiles[:])
            x0_sb = cp.tile([128, NT * 64], F16, tag="x0_sb")
            nc.sync.dma_start(out=x0_sb[:], in_=x0t[:])

            psum_rew = None
            if variant not in ("ccband", "norew"):
                psum_rew = prp.tile([128, 512], F32, tag="rew")

            for rep in range(n_repeat):
              xt_prev = x0_sb
              for t in range(l_steps):
                  # ---- mask prefetch (replicated across partitions) ----
                  mstep = lp.tile([128, A * NT * 64], F16, tag="mstep")
                  nc.gpsimd.dma_start(out=mstep[:], in_=masks[t])

                  # ---- xhat: action-masked X^T copies ----
                  xhat = lp.tile([128, A * NT * 64], F16, tag="xhat")
                  for a in (range(A) if variant != "ccband" else ()):
                      nc.vector.tensor_tensor(
                          out=xhat[:, a * 256:(a + 1) * 256],
                          in0=xt_prev[:, 0:256],
                          in1=mstep[:, a * 256:(a + 1) * 256],
                          op=mybir.AluOpType.mult,
                      )

                  # ---- main sweep: next-interface slice, 4 actions on 4 PE
                  #      column groups ----
                  pm = pmp.tile([128, NS], F32, tag="pm")
                  if variant != "ccband":
                      for kt in range(KT):
                          for a in range(A):
                              r_, nt_ = kt // NT, kt % NT
                              lhsT = xhat[:, a * 256 + nt_ * 64 + r_ * 8: a * 256 + nt_ * 64 + r_ * 8 + 8]
                              nc.tensor.matmul(
                                  out=pm[32 * a:32 * a + 8, :],
                                  lhsT=lhsT,
                                  rhs=t_sb[:, (a * KT + kt) * NS:(a * KT + kt + 1) * NS],
                                  start=(kt == 0),
                                  stop=(kt == KT - 1),
                                  tile_position=(0, 32 * a),
                              )

                  # ---- reward pass (fills the PE during the collective) ----
                  if variant not in ("norew", "ccband"):
                      for kt in range(KT):
                          for a in range(A):
                              r_, nt_ = kt // NT, kt % NT
                              lhsT = xhat[:, a * 256 + nt_ * 64 + r_ * 8: a * 256 + nt_ * 64 + r_ * 8 + 8]
                              nc.tensor.matmul(
                                  out=psum_rew[32 * a:32 * a + 8, rep * l_steps + t:rep * l_steps + t + 1],
                                  lhsT=lhsT,
                                  rhs=r_sb[:, a * KT + kt:a * KT + kt + 1],
                                  start=(kt == 0),
                                  stop=(kt == KT - 1),
                                  tile_position=(0, 32 * a),
                                  skip_group_check=True,
                              )

                  # ---- junk warmth: keep the PE HAM-warm during the exchange ----
                  if n_junk > 0 and t < l_steps - 1:
                      pj = pjp.tile([128, NS], F32, tag="pj")
                      for j in range(n_junk):
                          nc.tensor.matmul(
                              out=pj[0:8, :],
                              lhsT=xhat[:, 0:8],
                              rhs=t_sb[:, (j % 128) * NS:(j % 128 + 1) * NS],
                              start=True, stop=True,
                              tile_position=(0, 0),
                              skip_group_check=True,
                          )

                  # ---- evacuate + fold the 4 column groups; cast to fp16 ----
                  bounce = lp.tile([B, NS], F16, tag="bounce")
                  if variant == "ccband":
                      nc.vector.tensor_copy(out=bounce[:, 0:256], in_=xt_prev[:8, 0:256])
                  else:
                      c0 = lp.tile([B, NS], F32, tag="c0")
                      nc.vector.tensor_copy(out=c0[:], in_=pm[0:8, :])
                      c1 = lp.tile([B, NS], F32, tag="c1")
                      nc.vector.tensor_add(out=c1[:], in0=c0[:], in1=pm[32:40, :])
                      c2 = lp.tile([B, NS], F32, tag="c2")
                      nc.vector.tensor_add(out=c2[:], in0=c1[:], in1=pm[64:72, :])
                      nc.vector.tensor_add(out=bounce[:], in0=c2[:], in1=pm[96:104, :])

                  if t == l_steps - 1:
                      break

                  if variant == "nocc":
                      continue
                  # ---- exchange the interface slices ----
                  pp = t % 2
                  if variant == "noag":
                      nc.gpsimd.dma_start(out=cc_out[pp][0:8, :], in_=bounce[:])
                  else:
                      nc.gpsimd.dma_start(out=cc_in[pp][:], in_=bounce[:])
                      nc.gpsimd.collective_compute(
                          "AllGather",
                          mybir.AluOpType.bypass,
                          replica_groups=[list(range(N_CORES))],
                          ins=[cc_in[pp][:]],
                          outs=[cc_out[pp][:]],
                      )
                  xt = lp.tile([128, NT * 64], F16, tag="xt")
                  nc.sync.dma_start(
                      out=xt[:].rearrange("p (di m) -> p di m", di=NT),
                      in_=cc_out[pp][:].rearrange("m (di do) -> m di do", do=128),
                      transpose=True,
                  )
                  xt_prev = xt

            # ---- final: fold reward column groups, store ----
            if variant in ("ccband", "norew"):
                zf = cp.tile([B, l_steps], F32, tag="zf")
                nc.vector.memset(zf[:], 0.0)
                nc.gpsimd.dma_start(out=out[:], in_=zf[:])
                raise_skip = True
            else:
                raise_skip = False
            r0 = cp.tile([B, l_steps], F32, tag="r0")
            if not raise_skip:
                nc.vector.tensor_copy(out=r0[:], in_=psum_rew[0:8, 0:l_steps])
            if not raise_skip:
                r1 = cp.tile([B, l_steps], F32, tag="r1")
                nc.vector.tensor_add(out=r1[:], in0=r0[:], in1=psum_rew[32:40, 0:l_steps])
                r2 = cp.tile([B, l_steps], F32, tag="r2")
                nc.vector.tensor_add(out=r2[:], in0=r1[:], in1=psum_rew[64:72, 0:l_steps])
                rfin = cp.tile([B, l_steps], F32, tag="rfin")
                nc.vector.tensor_add(out=rfin[:], in0=r2[:], in1=psum_rew[96:104, 0:l_steps])
                nc.gpsimd.dma_start(out=out[:], in_=rfin[:])

    _split_waits(nc, mybir)
    return nc


def _build_split2(l_steps: int, n_repeat: int = 1):
    """Split the sweep into two 256-col chunks (separate PSUM banks) so each
    chunk's AllGather overlaps the other chunk's matmuls, and order the
    k-tiles so the next sweep starts on chunk-A data while chunk-B's gather
    is still in flight."""
    from concourse import bass, tile
    from concourse.bass import mybir

    F32 = mybir.dt.float32
    F16 = mybir.dt.float16
    HC = NS // 2  # 256: chunk width

    nc = bass.Bass(num_devices=N_CORES)

    t_tiles = nc.declare_dram_parameter("t_tiles", [128, A * KT * NS], F16, isOutput=False)
    r_tiles = nc.declare_dram_parameter("r_tiles", [128, A * KT], F16, isOutput=False)
    x0t = nc.declare_dram_parameter("x0t", [128, NT * 64], F16, isOutput=False)
    masks = nc.declare_dram_parameter("masks", [l_steps, 128, A * NT * 64], F16, isOutput=False)
    out = nc.declare_dram_parameter("out", [B, l_steps], F32, isOutput=True)

    # 2 halves x 2 parities of collective buffers
    cc_in = [nc.dram_tensor(f"cc_in{i}", [B, HC], F16) for i in range(4)]
    cc_out = [
        nc.dram_tensor(f"cc_out{i}", [N_CORES * B, HC], F16, addr_space="Shared")
        for i in range(4)
    ]

    # k-tile order: tiles covered by chunk-A gathers first (nt 0,1)
    kts_af = [kt for kt in range(KT) if kt % NT < 2]
    kts_bf = [kt for kt in range(KT) if kt % NT >= 2]
    kt_order = kts_af + kts_bf

    def lhs_slice(xh, a, kt):
        # xh half-tile layout: [128, a*128 + (nt%2)*64 + r*8]
        r_, nt_ = kt // NT, kt % NT
        c = a * 128 + (nt_ % 2) * 64 + r_ * 8
        return xh[:, c:c + 8]

    with tile.TileContext(nc) as tc:
        with tc.tile_pool(name="const", bufs=1) as cp, \
             tc.tile_pool(name="loop", bufs=3) as lp, \
             tc.tile_pool(name="psA", bufs=2, space="PSUM") as ppa, \
             tc.tile_pool(name="psB", bufs=2, space="PSUM") as ppb, \
             tc.tile_pool(name="psr", bufs=1, space="PSUM") as prp:

            t_sb = cp.tile([128, A * KT * NS], F16, tag="t_sb")
            nc.sync.dma_start(out=t_sb[:], in_=t_tiles[:])
            r_sb = cp.tile([128, A * KT], F16, tag="r_sb")
            nc.sync.dma_start(out=r_sb[:], in_=r_tiles[:])
            x0_sb = cp.tile([128, NT * 64], F16, tag="x0_sb")
            nc.sync.dma_start(out=x0_sb[:], in_=x0t[:])

            psum_rew = prp.tile([128, 512], F32, tag="rew")

            for rep in range(n_repeat):
                xta_prev, xtb_prev = x0_sb[:, 0:128], x0_sb[:, 128:256]
                for t in range(l_steps):
                    mstep = lp.tile([128, A * NT * 64], F16, tag="mstep")
                    nc.gpsimd.dma_start(out=mstep[:], in_=masks[t])

                    # masked X^T halves
                    xha = lp.tile([128, A * 128], F16, tag="xha")
                    xhb = lp.tile([128, A * 128], F16, tag="xhb")
                    for a in range(A):
                        nc.vector.tensor_tensor(
                            out=xha[:, a * 128:(a + 1) * 128],
                            in0=xta_prev,
                            in1=mstep[:, a * 256:a * 256 + 128],
                            op=mybir.AluOpType.mult,
                        )
                    for a in range(A):
                        nc.vector.tensor_tensor(
                            out=xhb[:, a * 128:(a + 1) * 128],
                            in0=xtb_prev,
                            in1=mstep[:, a * 256 + 128:a * 256 + 256],
                            op=mybir.AluOpType.mult,
                        )

                    pmA = ppa.tile([128, HC], F32, tag="pmA")
                    pmB = ppb.tile([128, HC], F32, tag="pmB")
                    last = t == l_steps - 1
                    pp = t % 2

                    for half, pm in ((0, pmA), (1, pmB)):
                        off = half * HC
                        for i, kt in enumerate(kt_order):
                            xh = xha if kt % NT < 2 else xhb
                            for a in range(A):
                                nc.tensor.matmul(
                                    out=pm[32 * a:32 * a + 8, :],
                                    lhsT=lhs_slice(xh, a, kt),
                                    rhs=t_sb[:, (a * KT + kt) * NS + off:
                                             (a * KT + kt) * NS + off + HC],
                                    start=(i == 0),
                                    stop=(i == KT - 1),
                                    tile_position=(0, 32 * a),
                                )
                        # evacuate + fold this chunk, then kick its gather
                        c0 = lp.tile([B, HC], F32, tag=f"c0{half}")
                        nc.vector.tensor_copy(out=c0[:], in_=pm[0:8, :])
                        c1 = lp.tile([B, HC], F32, tag=f"c1{half}")
                        nc.vector.tensor_add(out=c1[:], in0=c0[:], in1=pm[32:40, :])
                        c2 = lp.tile([B, HC], F32, tag=f"c2{half}")
                        nc.vector.tensor_add(out=c2[:], in0=c1[:], in1=pm[64:72, :])
                        bounce = lp.tile([B, HC], F16, tag=f"bounce{half}")
                        nc.vector.tensor_add(out=bounce[:], in0=c2[:], in1=pm[96:104, :])
                        if not last:
                            buf = 2 * pp + half
                            nc.gpsimd.dma_start(out=cc_in[buf][:], in_=bounce[:])
                            nc.gpsimd.collective_compute(
                                "AllGather",
                                mybir.AluOpType.bypass,
                                replica_groups=[list(range(N_CORES))],
                                ins=[cc_in[buf][:]],
                                outs=[cc_out[buf][:]],
                            )

                    # reward pass (overlaps the exchanges)
                    for kt in kt_order:
                        xh = xha if kt % NT < 2 else xhb
                        for a in range(A):
                            nc.tensor.matmul(
                                out=psum_rew[32 * a:32 * a + 8,
                                             rep * l_steps + t:rep * l_steps + t + 1],
                                lhsT=lhs_slice(xh, a, kt),
                                rhs=r_sb[:, a * KT + kt:a * KT + kt + 1],
                                start=(kt == kt_order[0]),
                                stop=(kt == kt_order[-1]),
                                tile_position=(0, 32 * a),
                                skip_group_check=True,
                            )

                    if last:
                        break

                    # transposes: rebuild X^T halves for the next step
                    xta = lp.tile([128, 2 * 64], F16, tag="xta")
                    nc.sync.dma_start(
                        out=xta[:].rearrange("p (di m) -> p di m", di=2),
                        in_=cc_out[2 * pp][:].rearrange("m (di do) -> m di do", do=128),
                        transpose=True,
                    )
                    xtb = lp.tile([128, 2 * 64], F16, tag="xtb")
                    nc.sync.dma_start(
                        out=xtb[:].rearrange("p (di m) -> p di m", di=2),
                        in_=cc_out[2 * pp + 1][:].rearrange("m (di do) -> m di do", do=128),
                        transpose=True,
                    )
                    xta_prev, xtb_prev = xta[:, :], xtb[:, :]

            # final: fold reward column groups, store
            r0 = cp.tile([B, l_steps], F32, tag="r0")
            nc.vector.tensor_copy(out=r0[:], in_=psum_rew[0:8, 0:l_steps])
            r1 = cp.tile([B, l_steps], F32, tag="r1")
            nc.vector.tensor_add(out=r1[:], in0=r0[:], in1=psum_rew[32:40, 0:l_steps])
            r2 = cp.tile([B, l_steps], F32, tag="r2")
            nc.vector.tensor_add(out=r2[:], in0=r1[:], in1=psum_rew[64:72, 0:l_steps])
            rfin = cp.tile([B, l_steps], F32, tag="rfin")
            nc.vector.tensor_add(out=rfin[:], in0=r2[:], in1=psum_rew[96:104, 0:l_steps])
            nc.gpsimd.dma_start(out=out[:], in_=rfin[:])

    _split_waits(nc, mybir)
    return nc


def _split_waits(nc, mybir, max_waits: int = 1):
    """Walrus rejects >1 sem wait on DMA/CTRL structs; spill extras to NoOps."""
    for bb in nc.main_func.blocks:
        insts = list(bb.instructions)
        new = []
        changed = False
        for ins in insts:
            si = getattr(ins, "sync_info", None)
            if si is not None and len(si.on_wait) > max_waits:
                waits = list(si.on_wait)
                for k, w in enumerate(waits[:-max_waits]):
                    new.append(
                        mybir.InstNoOp(
                            name=f"{ins.name}-wsplit{k}",
                            sync_info=mybir.SyncInfo(on_wait=[w], on_update=[]),
                            bass_nofuse=True,
                            engine=ins.engine,
                        )
                    )
                ins.sync_info = mybir.SyncInfo(
                    on_wait=waits[-max_waits:], on_update=list(si.on_update)
                )
                changed = True
            new.append(ins)
        if changed:
            live = bb.instructions
            live[:] = new


def _prepare_inputs(init_states, actions, T, R, l_steps):
    init_states = np.asarray(init_states).astype(np.int64)
    actions = np.asarray(actions).astype(np.int64)
    T = np.asarray(T, dtype=np.float32)
    R = np.asarray(R, dtype=np.float32)

    T16 = T.astype(np.float16)
    R16 = R.astype(np.float16)

    # r_tiles[p, a*KT + kt] = R[a, kt*128 + p]
    r_tiles = np.ascontiguousarray(
        R16.reshape(A, KT, 128).transpose(2, 0, 1).reshape(128, A * KT)
    )

    # x0t[p, nt*64 + r*8 + b] = X0[b, r*512 + nt*128 + p]
    x0t = np.zeros((128, NT, 64), dtype=np.float16)
    for b in range(B):
        s0 = int(init_states[b])
        r_, rem = divmod(s0, NS)
        nt_, p = divmod(rem, 128)
        x0t[p, nt_, r_ * 8 + b] = 1.0
    x0t = x0t.reshape(128, NT * 64)

    # masks[t, p, a*256 + nt*64 + r*8 + b] = (actions[b, t] == a), all p
    lfull = actions.shape[1]
    onehot = (actions.T[:, None, :] == np.arange(A)[None, :, None])  # [L, A, B]
    masks = np.broadcast_to(
        onehot[:, None, :, None, None, :], (lfull, 128, A, NT, N_CORES, B)
    ).astype(np.float16).reshape(lfull, 128, A * NT * 64)
    masks = np.ascontiguousarray(masks[:l_steps])

    # per-core T tiles: t_tiles[p, (a*KT + kt)*NS + n] = T[a, kt*128+p, r*NS+n]
    in_maps = []
    for r_ in range(N_CORES):
        tc_ = T16[:, :, r_ * NS:(r_ + 1) * NS]                 # [A, S, NS]
        tt = np.ascontiguousarray(
            tc_.reshape(A, KT, 128, NS).transpose(2, 0, 1, 3).reshape(128, A * KT * NS)
        )
        in_maps.append({
            "t_tiles": tt,
            "r_tiles": r_tiles,
            "x0t": x0t,
            "masks": masks,
        })
    return in_maps


def _run(init_states, actions, T, R, l_steps=L, trace=False):
    from concourse.bass_utils import run_bass_kernel_spmd

    import os as _os
    variant = _os.environ.get("KVARIANT", "full")
    n_repeat = int(_os.environ.get("KREPEAT", "1"))
    n_junk = int(_os.environ.get("KJUNK", "0"))
    key = (l_steps, variant, n_repeat, n_junk)
    if key not in _CACHE:
        if variant == "split2":
            _CACHE[key] = _build_split2(l_steps, n_repeat)
        else:
            _CACHE[key] = _build(l_steps, variant, n_repeat, n_junk)
    nc = _CACHE[key]
    in_maps = _prepare_inputs(init_states, actions, T, R, l_steps)
    res = run_bass_kernel_spmd(
        nc, in_maps, list(range(N_CORES)), trace=trace
    )
    rewards = res.results[0]["out"].astype(np.float32)
    return rewards, res


def kernel(init_states, actions, T, R):
    rewards, _ = _run(init_states, actions, T, R, l_steps=L, trace=False)
    return rewards

